# revision 1
# baseline (speedup 1.0000x reference)
"""Bass/Tile TRN2 kernel for nn_Attention_12704513261709.

8-way head-parallel attention: each of the 8 NeuronCores computes one head
(dh = 2048) over both batches, plus its partial (row-parallel) output
projection. Host sums the 8 partials.

Shapes (from reference.setup_inputs):
  x: (2, 2048, 256) f32, gamma: (256,), Wq/Wk/Wv: (16384, 256), Wo: (256, 16384)
"""

import numpy as np
import ml_dtypes

B = 2
N_SEQ = 2048
N_TOK = B * N_SEQ  # 4096
D = 256
HEADS = 8
INNER = 16384
DH = INNER // HEADS  # 2048
SCALE = 64 ** (-0.5)
EPS = 1e-5

FT = DH // 128  # 16 f-tiles per head dim
TT = N_SEQ // 128  # 16 key tiles per batch
NCH = N_SEQ // 512  # 4 query chunks of 512 per batch

_CACHE = {}


def _build():
    from concourse import bacc, bass_isa
    import concourse.tile as tile
    import concourse.mybir as mybir
    from concourse.masks import make_identity

    f32 = mybir.dt.float32
    bf16 = mybir.dt.bfloat16
    AF = mybir.ActivationFunctionType
    ALU = mybir.AluOpType

    nc = bacc.Bacc("TRN2", target_bir_lowering=False, debug=False, num_devices=8)

    x_d = nc.dram_tensor("x", [N_TOK, D], f32, kind="ExternalInput").ap()
    wqT_d = nc.dram_tensor("wqT", [D, DH], bf16, kind="ExternalInput").ap()
    wkT_d = nc.dram_tensor("wkT", [D, DH], bf16, kind="ExternalInput").ap()
    wvT_d = nc.dram_tensor("wvT", [D, DH], bf16, kind="ExternalInput").ap()
    woT_d = nc.dram_tensor("woT", [DH, D], bf16, kind="ExternalInput").ap()
    out_d = nc.dram_tensor("outT", [D, N_TOK], f32, kind="ExternalOutput").ap()

    with tile.TileContext(nc) as tc:
        with (
            tc.tile_pool(name="singles", bufs=1) as singles,
            tc.tile_pool(name="ln", bufs=4) as ln_pool,
            tc.tile_pool(name="big", bufs=1) as big,
            tc.tile_pool(name="qt", bufs=1) as qt_pool,
            tc.tile_pool(name="pt", bufs=1) as pt_pool,
            tc.tile_pool(name="ot", bufs=1) as ot_pool,
            tc.tile_pool(name="vstrip", bufs=5) as vs_pool,
            tc.tile_pool(name="stage", bufs=2) as stage_pool,
            tc.tile_pool(name="rsum", bufs=1) as rsum_pool,
            tc.tile_pool(name="dram", bufs=2, space="DRAM") as dram_pool,
            tc.tile_pool(name="psA", bufs=4, space="PSUM") as psA,
            tc.tile_pool(name="psB", bufs=2, space="PSUM") as psB,
            tc.tile_pool(name="psM", bufs=2, space="PSUM") as psM,
        ):
            identity = singles.tile([128, 128], f32)
            make_identity(nc, identity)
            eps_t = singles.tile([128, 1], f32)
            nc.vector.memset(eps_t, EPS)
            warm = singles.tile([128, 1], f32)
            nc.scalar.activation(warm[:], eps_t[:], func=AF.Sqrt, bias=eps_t[:], scale=1.0)
            # dummy matmuls fill the pre-work window (x DMA + LN chain latency)
            # so the HAM clock-gate is already at 8/8 when real matmuls arrive
            dummy_w = singles.tile([128, 128], bf16)
            nc.vector.memset(dummy_w, 0.0)
            dummy_r = singles.tile([128, 256], bf16)
            nc.vector.memset(dummy_r, 0.0)
            for _ in range(32):
                ps = psM.tile([128, 512], f32, tag="m", name="hamwarm")
                nc.tensor.matmul(ps[:, :256], dummy_w[:], dummy_r[:], start=True, stop=True)

            # weights to SBUF
            wqT = [big.tile([128, DH], bf16, tag=f"wq{d_}", name=f"wq{d_}") for d_ in range(2)]
            wkT = [big.tile([128, DH], bf16, tag=f"wk{d_}", name=f"wk{d_}") for d_ in range(2)]
            wvT = [big.tile([128, DH], bf16, tag=f"wv{d_}", name=f"wv{d_}") for d_ in range(2)]
            woT = [big.tile([128, D], bf16, tag=f"wo{fc}", name=f"wo{fc}") for fc in range(FT)]

            xnT = [big.tile([128, N_TOK], bf16, tag=f"xnT{d_}", name=f"xnT{d_}") for d_ in range(2)]

            state = {}

            def ln_chain(i):
                """LayerNorm token tile i (128 tokens): DVE/ACT chain only."""
                x_t = ln_pool.tile([128, D], f32, tag="x", name="x")
                nc.sync.dma_start(x_t[:], x_d[i * 128 : (i + 1) * 128, :])
                stats = ln_pool.tile([128, nc.vector.BN_STATS_DIM], f32, tag="st", name="st")
                nc.vector.bn_stats(stats[:], x_t[:])
                mv = ln_pool.tile([128, nc.vector.BN_AGGR_DIM], f32, tag="mv", name="mv")
                nc.vector.bn_aggr(mv[:], stats[:])
                std = ln_pool.tile([128, 1], f32, tag="std", name="std")
                nc.scalar.activation(
                    std[:], mv[:, 1:2], func=AF.Sqrt, bias=eps_t[:], scale=1.0
                )
                rstd = ln_pool.tile([128, 1], f32, tag="rstd", name="rstd")
                nc.vector.reciprocal(rstd[:], std[:])
                xn_t = ln_pool.tile([128, D], f32, tag="xn", name="xn")
                nc.vector.tensor_scalar(
                    xn_t[:],
                    x_t[:],
                    scalar1=mv[:, 0:1],
                    scalar2=rstd[:],
                    op0=ALU.subtract,
                    op1=ALU.mult,
                )
                state[f"xn{i % 8}"] = xn_t

            def ln_transpose(i):
                xn_t = state[f"xn{i % 8}"]
                for d_ in range(2):
                    ps = psM.tile([128, 512], f32, tag="m", name="m")
                    nc.tensor.transpose(
                        ps[:, :128], xn_t[:, d_ * 128 : (d_ + 1) * 128], identity[:]
                    )
                    nc.any.tensor_copy(xnT[d_][:, i * 128 : (i + 1) * 128], ps[:, :128])

            def ln_tile(i):
                ln_chain(i)
                ln_transpose(i)

            def kt_build_group(bb, nch):
                """K^T columns for one 512-token group of batch bb."""
                base = bb * N_SEQ
                for ft in range(FT):
                    ps = psM.tile([128, 512], f32, tag="m", name="m")
                    for d_ in range(2):
                        nc.tensor.matmul(
                            ps[:],
                            wkT[d_][:, ft * 128 : (ft + 1) * 128],
                            xnT[d_][:, base + nch * 512 : base + (nch + 1) * 512],
                            start=(d_ == 0),
                            stop=(d_ == 1),
                        )
                    nc.any.tensor_copy(
                        state[f"kt{ft}"][:, nch * 512 : (nch + 1) * 512], ps[:]
                    )

            def v_build_tile(bb, t):
                """V rows for key tile t of batch bb -> blocked DRAM scratch."""
                base = bb * N_SEQ
                v_stage = stage_pool.tile([128, DH], bf16, tag="vstage", name="vstage")
                for fch in range(4):
                    ps = psM.tile([128, 512], f32, tag="m", name="m")
                    for d_ in range(2):
                        nc.tensor.matmul(
                            ps[:],
                            xnT[d_][:, base + t * 128 : base + (t + 1) * 128],
                            wvT[d_][:, fch * 512 : (fch + 1) * 512],
                            start=(d_ == 0),
                            stop=(d_ == 1),
                        )
                    nc.any.tensor_copy(v_stage[:, fch * 512 : (fch + 1) * 512], ps[:])
                nc.sync.dma_start(
                    state["v_dram2"][:, :, t, :].rearrange("g p f -> p g f"),
                    v_stage.rearrange("p (g f) -> p g f", g=FT),
                )

            def qt_build(bb, ch):
                cbase = bb * N_SEQ + ch * 512
                state["qt"] = []
                for ft in range(FT):
                    ps = psA.tile([128, 512], f32, tag="st", name="qtps")
                    for d_ in range(2):
                        nc.tensor.matmul(
                            ps[:],
                            wqT[d_][:, ft * 128 : (ft + 1) * 128],
                            xnT[d_][:, cbase : cbase + 512],
                            start=(d_ == 0),
                            stop=(d_ == 1),
                        )
                    qt = qt_pool.tile([128, 512], bf16, tag=f"qt{ft}", name=f"qt{ft}")
                    nc.any.tensor_copy(qt[:], ps[:])
                    state["qt"].append(qt)

            def phase_a(bb, ch, extras=()):
                """S^T + exp per key tile; prefetch V strips. Rowsum is done
                entirely off the PE: DVE reduce over key tiles, GpSimd
                partition all-reduce (which also broadcasts), DVE reciprocal."""
                QT = state["qt"]
                KT = [state[f"kt{ft}"] for ft in range(FT)]
                pt_big = pt_pool.tile([128, TT * 512], bf16, tag="pt", name="pt")
                state["pt_big"] = pt_big
                state["pt"] = [
                    pt_big[:, t * 512 : (t + 1) * 512] for t in range(TT)
                ]
                state["strips"] = []
                for t in range(TT):
                    strip = vs_pool.tile([128, TT, 128], bf16, tag="vstrip", name="vstrip")
                    nc.gpsimd.dma_start(strip[:], state["v_dram"][t])
                    state["strips"].append(strip)
                    st_ps = psA.tile([128, 512], f32, tag="st", name="st")
                    for ft in range(FT):
                        nc.tensor.matmul(
                            st_ps[:],
                            KT[ft][:, t * 128 : (t + 1) * 128],
                            QT[ft][:],
                            start=(ft == 0),
                            stop=(ft == FT - 1),
                        )
                    nc.scalar.activation(state["pt"][t], st_ps[:], func=AF.Exp)
                    if t < len(extras):
                        extras[t]()
                for j in range(TT, len(extras)):
                    extras[j]()

            def phase_rsum():
                rsum_p = rsum_pool.tile([128, 512], f32, tag="rsum_p", name="rsum_p")
                nc.vector.tensor_reduce(
                    rsum_p[:],
                    state["pt_big"].rearrange("p (t i) -> p i t", t=TT),
                    axis=mybir.AxisListType.X,
                    op=ALU.add,
                )
                rsum_all = rsum_pool.tile([128, 512], f32, tag="rsum_a", name="rsum_a")
                nc.gpsimd.partition_all_reduce(
                    rsum_all[:], rsum_p[:], channels=128,
                    reduce_op=bass_isa.ReduceOp.add,
                )
                rbc = rsum_pool.tile([128, 512], f32, tag="rbc", name="rbc")
                nc.vector.reciprocal(rbc[:], rsum_all[:])
                state["rbc"] = rbc

            def phase_b(extras=(), fc_start=0, fc_stop=FT):
                PT = state["pt"]
                if fc_start == 0:
                    state["ot"] = []
                for fc in range(fc_start, fc_stop):
                    if fc % 3 == 2 and fc // 3 < len(extras):
                        extras[fc // 3]()
                    strip = state["strips"][fc]
                    ot_ps = psB.tile([128, 512], f32, tag="ot", name="ot")
                    for t in range(TT):
                        nc.tensor.matmul(
                            ot_ps[:],
                            strip[:, t, :],
                            PT[t][:],
                            start=(t == 0),
                            stop=(t == TT - 1),
                        )
                    ot = ot_pool.tile([128, 512], bf16, tag=f"ot{fc}", name=f"ot{fc}")
                    nc.any.tensor_copy(ot[:], ot_ps[:])
                    state["ot"].append(ot)
                for j in range(FT // 3, len(extras)):
                    extras[j]()

            def phase_c(bb, ch):
                cbase = bb * N_SEQ + ch * 512
                OT = state["ot"]
                for dm in range(2):
                    op_ps = psM.tile([128, 512], f32, tag="m", name="m")
                    for fc in range(FT):
                        nc.tensor.matmul(
                            op_ps[:],
                            woT[fc][:, dm * 128 : (dm + 1) * 128],
                            OT[fc][:],
                            start=(fc == 0),
                            stop=(fc == FT - 1),
                        )
                    op_sb = stage_pool.tile([128, 512], f32, tag="opsb", name="opsb")
                    nc.vector.tensor_tensor(
                        op_sb[:], op_ps[:], state["rbc"][:], ALU.mult
                    )
                    nc.sync.dma_start(
                        out_d[dm * 128 : (dm + 1) * 128, cbase : cbase + 512],
                        op_sb[:],
                    )

            def kt_alloc(bb):
                for ft in range(FT):
                    state[f"kt{ft}"] = big.tile(
                        [128, N_SEQ], bf16, tag=f"kt{ft}", name=f"kt{ft}"
                    )

            def v_alloc(bb):
                state["v_dram2"] = dram_pool.tile(
                    [FT, 128, TT, 128], bf16, tag="vscratch", name="vscratch"
                )

            def v_promote():
                state["v_dram"] = state["v_dram2"]

            # ---- prologue: batch-0 LN interleaved with batch-0 K/V builds,
            # V/KT lagging one tile so transpose->copy latency stays hidden ----
            kt_alloc(0)
            v_alloc(0)
            v_promote()
            for i in range(3):
                ln_chain(i)  # x loads go first on the sync queue
            nc.sync.dma_start(wvT[0][:], wvT_d[0:128, :])
            nc.gpsimd.dma_start(wvT[1][:], wvT_d[128:256, :])
            for d_ in range(2):
                sl = slice(d_ * 128, (d_ + 1) * 128)
                nc.gpsimd.dma_start(wkT[d_][:], wkT_d[sl, :])
                nc.gpsimd.dma_start(wqT[d_][:], wqT_d[sl, :])
            for fc in range(FT):
                nc.gpsimd.dma_start(woT[fc][:], woT_d[fc * 128 : (fc + 1) * 128, :])
            for i in range(TT):
                if i + 3 < TT:
                    ln_chain(i + 3)
                ln_transpose(i)
                if i > 0:
                    v_build_tile(0, i - 1)
                if i % 4 == 1 and i > 4:
                    kt_build_group(0, i // 4 - 1)
            v_build_tile(0, TT - 1)
            for g in (2, 3):
                kt_build_group(0, g)
            qt_build(0, 0)

            # ---- main loop over 8 chunks, with batch-1 prep woven into the
            # phase A/B matmul streams of batch-0 chunks ----
            def mk(f, *a):
                return lambda: f(*a)

            a_extras = {
                # batch-1 layernorm spread over chunks 0..2 (one per t-group)
                (0, 0): [mk(ln_tile, i) for i in range(TT, TT + 6)],
                # chunk 1: rest of LN + start batch-1 V (needs ln 16..21)
                (0, 1): [mk(ln_tile, i) for i in range(TT + 6, TT + 12)]
                + [mk(v_build_tile, 1, t) for t in range(0, 4)],
                (0, 2): [mk(ln_tile, i) for i in range(TT + 12, TT + 16)]
                + [mk(v_build_tile, 1, t) for t in range(4, 10)],
                (0, 3): [mk(v_build_tile, 1, t) for t in range(10, TT)],
            }
            b_extras = {
                # batch-1 K^T + next QT woven between B fc-groups of (0,3);
                # the copies only flow after A(0,3) reads finish, which holds
                (0, 3): [mk(kt_build_group, 1, g) for g in range(4)]
                + [mk(qt_build, 1, 0)],
            }
            for bb, ch in [(b_, c_) for b_ in range(B) for c_ in range(NCH)]:
                if (bb, ch) == (0, 1):
                    v_alloc(1)  # batch-1 scratch; strips still read batch-0's
                if (bb, ch) == (1, 0):
                    v_promote()  # batch-1 V scratch becomes current
                phase_a(bb, ch, a_extras.get((bb, ch), ()))
                if ch < NCH - 1:
                    qt_build(bb, ch + 1)
                phase_rsum()
                if (bb, ch) == (0, 3):
                    kt_alloc(1)
                phase_b(b_extras.get((bb, ch), ()))
                phase_c(bb, ch)

    nc.compile()
    return nc


def get_nc():
    if "nc" not in _CACHE:
        _CACHE["nc"] = _build()
    return _CACHE["nc"]


def make_in_maps(x, gamma, Wq, Wk, Wv, Wo):
    bf = ml_dtypes.bfloat16
    gp = (1.0 + gamma.astype(np.float64))[None, :]
    x_flat = np.ascontiguousarray(x.reshape(N_TOK, D).astype(np.float32))
    in_maps = []
    for h in range(HEADS):
        sl = slice(h * DH, (h + 1) * DH)
        wq = (Wq[sl].astype(np.float64) * gp * SCALE).T.astype(bf)
        wk = (Wk[sl].astype(np.float64) * gp).T.astype(bf)
        wv = (Wv[sl].astype(np.float64) * gp).T.astype(bf)
        wo = Wo[:, sl].T.astype(bf)
        in_maps.append(
            {
                "x": x_flat,
                "wqT": np.ascontiguousarray(wq),
                "wkT": np.ascontiguousarray(wk),
                "wvT": np.ascontiguousarray(wv),
                "woT": np.ascontiguousarray(wo),
            }
        )
    return in_maps


def kernel(x, gamma, Wq, Wk, Wv, Wo):
    from concourse import bass_utils

    x, gamma, Wq, Wk, Wv, Wo = (
        np.asarray(a) for a in (x, gamma, Wq, Wk, Wv, Wo)
    )
    nc = get_nc()
    in_maps = make_in_maps(x, gamma, Wq, Wk, Wv, Wo)
    res = bass_utils.run_bass_kernel_spmd(
        nc, in_maps, core_ids=list(range(HEADS))
    )
    acc = np.zeros((D, N_TOK), np.float32)
    for h in range(HEADS):
        acc += res.results[h]["outT"]
    return np.ascontiguousarray(acc.T).reshape(B, N_SEQ, D).astype(np.float32)



# revision 10
# speedup vs baseline: 3.9590x; 3.9590x over previous
"""Bass/Tile TRN2 kernel for nn_Attention_12704513261709 (low-rank factored).

Per-head dim (2048) >> model dim (256), so fold each head's weight pairs
into 256x256 matrices on the host:
  S_h = xn @ M_h @ xn^T    M_h = SCALE * diag(1+g) Wq_h^T Wk_h diag(1+g)
  Y_h = softmax(S_h) @ xn @ G_h    G_h = diag(1+g) Wv_h^T Wo_h^T
This cuts matmul FLOPs ~8.9x vs materializing q/k/v. Each of the 8 cores
computes one head over both batches; host sums the per-head partials.

Shapes (from reference.setup_inputs):
  x: (2, 2048, 256) f32, gamma: (256,), Wq/Wk/Wv: (16384, 256), Wo: (256, 16384)
"""

import numpy as np
import ml_dtypes

B = 2
N_SEQ = 2048
N_TOK = B * N_SEQ  # 4096
D = 256
HEADS = 8
INNER = 16384
DH = INNER // HEADS  # 2048
SCALE = 64 ** (-0.5)
EPS = 1e-5

TT = N_SEQ // 128  # 16 key tiles per batch
NCH = N_SEQ // 512  # 4 query chunks of 512 per batch
NTILE = N_TOK // 128  # 32 token tiles

_CACHE = {}


def _build():
    from concourse import bacc, bass_isa
    import concourse.tile as tile
    import concourse.mybir as mybir
    from concourse.masks import make_identity

    f32 = mybir.dt.float32
    bf16 = mybir.dt.bfloat16
    AF = mybir.ActivationFunctionType
    ALU = mybir.AluOpType

    nc = bacc.Bacc("TRN2", target_bir_lowering=False, debug=False, num_devices=8)

    x_d = nc.dram_tensor("x", [N_TOK, D], f32, kind="ExternalInput").ap()
    m_d = nc.dram_tensor("m", [D, D], bf16, kind="ExternalInput").ap()
    g_d = nc.dram_tensor("g", [D, D], bf16, kind="ExternalInput").ap()
    out_d = nc.dram_tensor("outT", [D, N_TOK], f32, kind="ExternalOutput").ap()

    with tile.TileContext(nc) as tc:
        with (
            tc.tile_pool(name="singles", bufs=1) as singles,
            tc.tile_pool(name="ln", bufs=4) as ln_pool,
            tc.tile_pool(name="xnf", bufs=6) as xnf_pool,
            tc.tile_pool(name="big", bufs=1) as big,
            tc.tile_pool(name="pt", bufs=2) as pt_pool,
            tc.tile_pool(name="ut", bufs=2) as ut_pool,
            tc.tile_pool(name="rsum", bufs=2) as rsum_pool,
            tc.tile_pool(name="ystage", bufs=2) as y_pool,
            tc.tile_pool(name="psA", bufs=4, space="PSUM") as psA,
            tc.tile_pool(name="psU", bufs=2, space="PSUM") as psU,
            tc.tile_pool(name="psM", bufs=2, space="PSUM") as psM,
        ):
            identity = singles.tile([128, 128], f32)
            make_identity(nc, identity)
            eps_t = singles.tile([128, 1], f32)
            nc.vector.memset(eps_t, EPS)
            # dummy matmuls fill the pre-work window (x DMA + LN chain latency)
            # so the HAM clock-gate is already at 8/8 when real matmuls arrive
            dummy_w = singles.tile([128, 128], bf16)
            nc.vector.memset(dummy_w, 0.0)
            dummy_r = singles.tile([128, 256], bf16)
            nc.vector.memset(dummy_r, 0.0)
            for _ in range(24):
                ps = psM.tile([128, 512], f32, tag="m", name="hamwarm")
                nc.tensor.matmul(ps[:, :256], dummy_w[:], dummy_r[:], start=True, stop=True)

            m_sb = [big.tile([128, D], bf16, tag=f"m{c}", name=f"m{c}") for c in range(2)]
            g_sb = [big.tile([128, D], bf16, tag=f"g{c}", name=f"g{c}") for c in range(2)]
            # xn in natural layout [token, d] (per 128-token tile) and
            # transposed [d, token]; T^T = (xn @ M)^T in [d, token]
            xn_nat = big.tile([128, NTILE * D], bf16, tag="xn", name="xn")
            xnT = [big.tile([128, N_TOK], bf16, tag=f"xnT{c}", name=f"xnT{c}") for c in range(2)]
            tT = [big.tile([128, N_TOK], bf16, tag=f"tT{c}", name=f"tT{c}") for c in range(2)]

            state = {}

            def ln_chain(t):
                """LayerNorm token tile t (128 tokens) -> xn_nat (bf16)."""
                x_t = ln_pool.tile([128, D], f32, tag="x", name="x")
                nc.sync.dma_start(x_t[:], x_d[t * 128 : (t + 1) * 128, :])
                stats = ln_pool.tile([128, nc.vector.BN_STATS_DIM], f32, tag="st", name="st")
                nc.vector.bn_stats(stats[:], x_t[:])
                mv = ln_pool.tile([128, nc.vector.BN_AGGR_DIM], f32, tag="mv", name="mv")
                nc.vector.bn_aggr(mv[:], stats[:])
                std = ln_pool.tile([128, 1], f32, tag="std", name="std")
                nc.scalar.activation(std[:], mv[:, 1:2], func=AF.Sqrt, bias=eps_t[:], scale=1.0)
                rstd = ln_pool.tile([128, 1], f32, tag="rstd", name="rstd")
                nc.vector.reciprocal(rstd[:], std[:])
                xn_f = xnf_pool.tile([128, D], f32, tag="xnf", name="xnf")
                nc.vector.tensor_scalar(
                    xn_f[:],
                    x_t[:],
                    scalar1=mv[:, 0:1],
                    scalar2=rstd[:],
                    op0=ALU.subtract,
                    op1=ALU.mult,
                )
                nc.gpsimd.tensor_copy(xn_nat[:, t * D : (t + 1) * D], xn_f[:])
                state[f"xnf{t}"] = xn_f

            def transpose_tile(t):
                xn_f = state[f"xnf{t}"]
                for c in range(2):
                    ps = psM.tile([128, 512], f32, tag="m", name="tr")
                    nc.tensor.transpose(
                        ps[:, :128], xn_f[:, c * 128 : (c + 1) * 128], identity[:]
                    )
                    nc.vector.tensor_copy(xnT[c][:, t * 128 : (t + 1) * 128], ps[:, :128])

            def tT_group(g):
                """T^T columns for one 512-token group: T^T = M^T-chunks @ xnT."""
                for c2 in range(2):
                    ps = psM.tile([128, 512], f32, tag="m", name="tT")
                    for c1 in range(2):
                        nc.tensor.matmul(
                            ps[:],
                            m_sb[c1][:, c2 * 128 : (c2 + 1) * 128],
                            xnT[c1][:, g * 512 : (g + 1) * 512],
                            start=(c1 == 0),
                            stop=(c1 == 1),
                        )
                    nc.vector.tensor_copy(tT[c2][:, g * 512 : (g + 1) * 512], ps[:])

            def phase_s(b, ch, extras=()):
                """S^T tiles + exp for one 512-query chunk: S^T[k,q] = xnT^T.T@tT."""
                cols = b * N_SEQ + ch * 512
                pt_big = pt_pool.tile([128, TT * 512], bf16, tag="pt", name="pt")
                state["pt"] = pt_big
                for t in range(TT):
                    ps = psA.tile([128, 512], f32, tag="s", name="s")
                    for c in range(2):
                        nc.tensor.matmul(
                            ps[:],
                            xnT[c][:, b * N_SEQ + t * 128 : b * N_SEQ + (t + 1) * 128],
                            tT[c][:, cols : cols + 512],
                            start=(c == 0),
                            stop=(c == 1),
                        )
                    nc.scalar.activation(pt_big[:, t * 512 : (t + 1) * 512], ps[:], func=AF.Exp)
                    if t < len(extras):
                        extras[t]()
                for j in range(TT, len(extras)):
                    extras[j]()

            def phase_rsum():
                """softmax denominator: DVE reduce over key tiles + GpSimd
                partition all-reduce (which also broadcasts), DVE reciprocal."""
                pt_big = state["pt"]
                rsum_p = rsum_pool.tile([128, 512], f32, tag="rp", name="rp")
                nc.vector.tensor_reduce(
                    rsum_p[:],
                    pt_big.rearrange("p (t i) -> p i t", t=TT),
                    axis=mybir.AxisListType.X,
                    op=ALU.add,
                )
                rsum_all = rsum_pool.tile([128, 512], f32, tag="ra", name="ra")
                nc.gpsimd.partition_all_reduce(
                    rsum_all[:], rsum_p[:], channels=128, reduce_op=bass_isa.ReduceOp.add
                )
                rbc = rsum_pool.tile([128, 512], f32, tag="rbc", name="rbc")
                nc.vector.reciprocal(rbc[:], rsum_all[:])
                state["rbc"] = rbc

            def phase_u(b, ch):
                """U^T[e,q] = sum_k xn[k,e] P^T[k,q], accumulated over 16 k-tiles."""
                pt_big = state["pt"]
                state["ut"] = []
                for e in range(2):
                    ps = psU.tile([128, 512], f32, tag="u", name="u")
                    for t in range(TT):
                        base = (b * TT + t) * D
                        nc.tensor.matmul(
                            ps[:],
                            xn_nat[:, base + e * 128 : base + (e + 1) * 128],
                            pt_big[:, t * 512 : (t + 1) * 512],
                            start=(t == 0),
                            stop=(t == TT - 1),
                        )
                    ut = ut_pool.tile([128, 512], bf16, tag=f"ut{e}", name=f"ut{e}")
                    nc.vector.tensor_copy(ut[:], ps[:])
                    state["ut"].append(ut)

            def phase_y(b, ch, ut, rbc):
                """Y^T[d2,q] = sum_e G[e,d2] U^T[e,q], scaled by 1/rowsum."""
                cols = b * N_SEQ + ch * 512
                for c2 in range(2):
                    ps = psU.tile([128, 512], f32, tag="u", name="y")
                    for e in range(2):
                        nc.tensor.matmul(
                            ps[:],
                            g_sb[e][:, c2 * 128 : (c2 + 1) * 128],
                            ut[e][:],
                            start=(e == 0),
                            stop=(e == 1),
                        )
                    y_sb = y_pool.tile([128, 512], f32, tag="y", name="y")
                    nc.vector.tensor_tensor(y_sb[:], ps[:], rbc[:], ALU.mult)
                    nc.sync.dma_start(out_d[c2 * 128 : (c2 + 1) * 128, cols : cols + 512], y_sb[:])

            # ---- prologue: weights + batch-0 LN/transpose/T^T, pipelined ----
            nc.gpsimd.dma_start(m_sb[0][:], m_d[0:128, :])
            nc.gpsimd.dma_start(m_sb[1][:], m_d[128:256, :])
            nc.gpsimd.dma_start(g_sb[0][:], g_d[0:128, :])
            nc.gpsimd.dma_start(g_sb[1][:], g_d[128:256, :])

            for t in range(3):
                ln_chain(t)
            for t in range(TT):
                if t + 3 < TT:
                    ln_chain(t + 3)
                transpose_tile(t)
                if t % 4 == 3:
                    tT_group(t // 4)

            def mk(f, *a):
                return lambda: f(*a)

            # batch-1 prep woven into batch-0's S-phase matmul streams,
            # LN leading its transpose by 4-5 tiles (xnf pool holds 6)
            s_extras = {
                (0, 0): [mk(ln_chain, t) for t in range(16, 21)]
                + [mk(transpose_tile, 16), mk(ln_chain, 21), mk(transpose_tile, 17),
                   mk(ln_chain, 22), mk(transpose_tile, 18), mk(ln_chain, 23),
                   mk(transpose_tile, 19), mk(tT_group, 4)],
                (0, 1): [mk(ln_chain, 24), mk(transpose_tile, 20), mk(ln_chain, 25),
                         mk(transpose_tile, 21), mk(ln_chain, 26), mk(transpose_tile, 22),
                         mk(ln_chain, 27), mk(transpose_tile, 23), mk(tT_group, 5)],
                (0, 2): [mk(ln_chain, 28), mk(transpose_tile, 24), mk(ln_chain, 29),
                         mk(transpose_tile, 25), mk(ln_chain, 30), mk(transpose_tile, 26),
                         mk(ln_chain, 31), mk(transpose_tile, 27), mk(tT_group, 6)],
                (0, 3): [mk(transpose_tile, t) for t in range(28, 32)]
                + [mk(tT_group, 7)],
            }

            pending_y = None
            for b in range(B):
                for ch in range(NCH):
                    extras = ([pending_y] if pending_y else []) + list(
                        s_extras.get((b, ch), ())
                    )
                    phase_s(b, ch, extras)
                    phase_rsum()
                    phase_u(b, ch)
                    pending_y = mk(phase_y, b, ch, state["ut"], state["rbc"])
            pending_y()

    nc.compile()
    return nc


def get_nc():
    if "nc" not in _CACHE:
        _CACHE["nc"] = _build()
    return _CACHE["nc"]


def make_in_maps(x, gamma, Wq, Wk, Wv, Wo):
    bf = ml_dtypes.bfloat16
    gp = 1.0 + gamma.astype(np.float64)
    x_flat = np.ascontiguousarray(x.reshape(N_TOK, D).astype(np.float32))
    Wq = Wq.astype(np.float64)
    Wk = Wk.astype(np.float64)
    Wv = Wv.astype(np.float64)
    Wo = Wo.astype(np.float64)
    in_maps = []
    for h in range(HEADS):
        sl = slice(h * DH, (h + 1) * DH)
        M = SCALE * (gp[:, None] * Wq[sl].T) @ (Wk[sl] * gp[None, :])
        G = (gp[:, None] * Wv[sl].T) @ Wo[:, sl].T
        in_maps.append(
            {
                "x": x_flat,
                "m": np.ascontiguousarray(M.astype(bf)),
                "g": np.ascontiguousarray(G.astype(bf)),
            }
        )
    return in_maps


def kernel(x, gamma, Wq, Wk, Wv, Wo):
    from concourse import bass_utils

    x, gamma, Wq, Wk, Wv, Wo = (
        np.asarray(a) for a in (x, gamma, Wq, Wk, Wv, Wo)
    )
    nc = get_nc()
    in_maps = make_in_maps(x, gamma, Wq, Wk, Wv, Wo)
    res = bass_utils.run_bass_kernel_spmd(
        nc, in_maps, core_ids=list(range(HEADS))
    )
    acc = np.zeros((D, N_TOK), np.float32)
    for h in range(HEADS):
        acc += res.results[h]["outT"]
    return np.ascontiguousarray(acc.T).reshape(B, N_SEQ, D).astype(np.float32)


# revision 11
# speedup vs baseline: 3.9892x; 1.0076x over previous
"""Bass/Tile TRN2 kernel for nn_Attention_12704513261709 (low-rank factored).

Per-head dim (2048) >> model dim (256), so fold each head's weight pairs
into 256x256 matrices on the host:
  S_h = xn @ M_h @ xn^T    M_h = SCALE * diag(1+g) Wq_h^T Wk_h diag(1+g)
  Y_h = softmax(S_h) @ xn @ G_h    G_h = diag(1+g) Wv_h^T Wo_h^T
This cuts matmul FLOPs ~8.9x vs materializing q/k/v. Each of the 8 cores
computes one head over both batches; host sums the per-head partials.

Perf notes: the PE p-state ramp (1.2 GHz until ~3us of continuous busy,
then 2.4 GHz) means the matmul stream must never stall: transposes are
XBAR DMA-transposes (not PE), LN sqrt/recip are batched per 8 tiles (so
the ACT table isn't thrashed between Sqrt and Exp), the softmax rowsum
is a contiguous bf16 add-ladder on DVE (a strided tensor_reduce is ~3x
slower), and batch-1 prep is woven into batch-0's S-phase streams.
"""

import numpy as np
import ml_dtypes

B = 2
N_SEQ = 2048
N_TOK = B * N_SEQ  # 4096
D = 256
HEADS = 8
INNER = 16384
DH = INNER // HEADS  # 2048
SCALE = 64 ** (-0.5)
EPS = 1e-5

TT = N_SEQ // 128  # 16 key tiles per batch
NCH = N_SEQ // 512  # 4 query chunks of 512 per batch
NTILE = N_TOK // 128  # 32 token tiles

_CACHE = {}


def _build():
    from concourse import bacc, bass_isa
    import concourse.tile as tile
    import concourse.mybir as mybir

    f32 = mybir.dt.float32
    bf16 = mybir.dt.bfloat16
    AF = mybir.ActivationFunctionType
    ALU = mybir.AluOpType

    nc = bacc.Bacc("TRN2", target_bir_lowering=False, debug=False, num_devices=8)

    x_d = nc.dram_tensor("x", [N_TOK, D], f32, kind="ExternalInput").ap()
    m_d = nc.dram_tensor("m", [D, D], bf16, kind="ExternalInput").ap()
    g_d = nc.dram_tensor("g", [D, D], bf16, kind="ExternalInput").ap()
    out_d = nc.dram_tensor("outT", [D, N_TOK], f32, kind="ExternalOutput").ap()

    with tile.TileContext(nc) as tc:
        with (
            tc.tile_pool(name="singles", bufs=1) as singles,
            tc.tile_pool(name="xt", bufs=20) as xt_pool,
            tc.tile_pool(name="lns", bufs=4) as lns_pool,
            tc.tile_pool(name="big", bufs=1) as big,
            tc.tile_pool(name="pt", bufs=2) as pt_pool,
            tc.tile_pool(name="ut", bufs=2) as ut_pool,
            tc.tile_pool(name="lad", bufs=1) as lad_pool,
            tc.tile_pool(name="rsum", bufs=2) as rsum_pool,
            tc.tile_pool(name="ystage", bufs=2) as y_pool,
            tc.tile_pool(name="psA", bufs=5, space="PSUM") as psA,
            tc.tile_pool(name="psU", bufs=2, space="PSUM") as psU,
            tc.tile_pool(name="psM", bufs=1, space="PSUM") as psM,
        ):
            eps_t = singles.tile([128, 1], f32)
            nc.vector.memset(eps_t, EPS)
            dummy_w = singles.tile([128, 128], bf16)
            nc.vector.memset(dummy_w, 0.0)
            dummy_r = singles.tile([128, 512], bf16)
            nc.vector.memset(dummy_r, 0.0)

            def warm(n):
                for _ in range(n):
                    ps = psM.tile([128, 512], f32, tag="m", name="hamwarm")
                    nc.tensor.matmul(ps[:], dummy_w[:], dummy_r[:], start=True, stop=True)

            m_sb = [big.tile([128, D], bf16, tag=f"m{c}", name=f"m{c}") for c in range(2)]
            g_sb = [big.tile([128, D], bf16, tag=f"g{c}", name=f"g{c}") for c in range(2)]
            # xn in natural layout [token, d] (bf16, per 128-token tile) and
            # transposed [d, token]; T^T = (xn @ M)^T in [d, token]
            xn_nat = big.tile([128, NTILE * D], bf16, tag="xn", name="xn")
            xnT = [big.tile([128, N_TOK], bf16, tag=f"xnT{c}", name=f"xnT{c}") for c in range(2)]
            tT = [big.tile([128, N_TOK], bf16, tag=f"tT{c}", name=f"tT{c}") for c in range(2)]
            mv_all = big.tile([128, NTILE, 2], f32, tag="mv", name="mv")
            rstd_all = big.tile([128, NTILE], f32, tag="rstd", name="rstd")

            state = {}

            def ln_stats(t):
                """x DMA + bn stats for token tile t; x tile kept for norm."""
                x_t = xt_pool.tile([128, D], f32, tag="x", name="x")
                nc.sync.dma_start(x_t[:], x_d[t * 128 : (t + 1) * 128, :])
                stats = lns_pool.tile([128, nc.vector.BN_STATS_DIM], f32, tag="st", name="st")
                nc.vector.bn_stats(stats[:], x_t[:])
                nc.vector.bn_aggr(mv_all[:, t, :], stats[:])
                state[f"x{t}"] = x_t

            def ln_finish8(g8):
                """sqrt+reciprocal for one 8-tile group (batched: 1 ACT op,
                1 DVE op instead of 8+8)."""
                std8 = lns_pool.tile([128, 8], f32, tag="std8", name="std8")
                nc.scalar.activation(
                    std8[:], mv_all[:, g8 * 8 : (g8 + 1) * 8, 1], func=AF.Sqrt,
                    bias=eps_t[:], scale=1.0,
                )
                nc.vector.reciprocal(rstd_all[:, g8 * 8 : (g8 + 1) * 8], std8[:])

            def ln_norm(t):
                nc.vector.tensor_scalar(
                    xn_nat[:, t * D : (t + 1) * D],
                    state[f"x{t}"][:],
                    scalar1=mv_all[:, t, 0:1],
                    scalar2=rstd_all[:, t : t + 1],
                    op0=ALU.subtract,
                    op1=ALU.mult,
                )

            def tr(t, eng):
                """XBAR DMA-transpose of token tile t into xnT (no PE work)."""
                for c in range(2):
                    eng.dma_start_transpose(
                        xnT[c][:, t * 128 : (t + 1) * 128],
                        xn_nat[:, t * D + c * 128 : t * D + (c + 1) * 128],
                    )

            def tT_group(g):
                """T^T columns for one 512-token group: T^T = M-chunks.T @ xnT."""
                for c2 in range(2):
                    ps = psU.tile([128, 512], f32, tag="u", name="tT")
                    for c1 in range(2):
                        nc.tensor.matmul(
                            ps[:],
                            m_sb[c1][:, c2 * 128 : (c2 + 1) * 128],
                            xnT[c1][:, g * 512 : (g + 1) * 512],
                            start=(c1 == 0),
                            stop=(c1 == 1),
                        )
                    nc.vector.tensor_copy(tT[c2][:, g * 512 : (g + 1) * 512], ps[:])

            def phase_s(b, ch, extras=()):
                """S^T tiles + exp for one 512-query chunk."""
                cols = b * N_SEQ + ch * 512
                pt_big = pt_pool.tile([128, TT * 512], bf16, tag="pt", name="pt")
                state["pt"] = pt_big
                for t in range(TT):
                    ps = psA.tile([128, 512], f32, tag="s", name="s")
                    for c in range(2):
                        nc.tensor.matmul(
                            ps[:],
                            xnT[c][:, b * N_SEQ + t * 128 : b * N_SEQ + (t + 1) * 128],
                            tT[c][:, cols : cols + 512],
                            start=(c == 0),
                            stop=(c == 1),
                        )
                    nc.scalar.activation(pt_big[:, t * 512 : (t + 1) * 512], ps[:], func=AF.Exp)
                    if t < len(extras):
                        extras[t]()
                for j in range(TT, len(extras)):
                    extras[j]()

            def phase_rsum():
                """softmax denominator: contiguous bf16 add-ladder on DVE,
                GpSimd partition all-reduce (also broadcasts), DVE reciprocal."""
                pt_big = state["pt"]
                r1 = lad_pool.tile([128, 4096], bf16, tag="r1", name="r1")
                nc.vector.tensor_tensor(r1[:], pt_big[:, :4096], pt_big[:, 4096:], ALU.add)
                r2 = lad_pool.tile([128, 2048], bf16, tag="r2", name="r2")
                nc.vector.tensor_tensor(r2[:], r1[:, :2048], r1[:, 2048:], ALU.add)
                r3 = lad_pool.tile([128, 1024], bf16, tag="r3", name="r3")
                nc.vector.tensor_tensor(r3[:], r2[:, :1024], r2[:, 1024:], ALU.add)
                r4 = lad_pool.tile([128, 512], f32, tag="r4", name="r4")
                nc.vector.tensor_tensor(r4[:], r3[:, :512], r3[:, 512:], ALU.add)
                ra = rsum_pool.tile([128, 512], f32, tag="ra", name="ra")
                nc.gpsimd.partition_all_reduce(
                    ra[:], r4[:], channels=128, reduce_op=bass_isa.ReduceOp.add
                )
                rbc = rsum_pool.tile([128, 512], f32, tag="rbc", name="rbc")
                nc.vector.reciprocal(rbc[:], ra[:])
                state["rbc"] = rbc

            def phase_u(b, ch):
                """U^T[e,q] = sum_k xn[k,e] P^T[k,q], accumulated over 16 k-tiles."""
                pt_big = state["pt"]
                state["ut"] = []
                for e in range(2):
                    ps = psU.tile([128, 512], f32, tag="u", name="u")
                    for t in range(TT):
                        base = (b * TT + t) * D
                        nc.tensor.matmul(
                            ps[:],
                            xn_nat[:, base + e * 128 : base + (e + 1) * 128],
                            pt_big[:, t * 512 : (t + 1) * 512],
                            start=(t == 0),
                            stop=(t == TT - 1),
                        )
                    ut = ut_pool.tile([128, 512], bf16, tag=f"ut{e}", name=f"ut{e}")
                    nc.vector.tensor_copy(ut[:], ps[:])
                    state["ut"].append(ut)

            def phase_y(b, ch, ut, rbc):
                """Y^T[d2,q] = sum_e G[e,d2] U^T[e,q], scaled by 1/rowsum."""
                cols = b * N_SEQ + ch * 512
                for c2 in range(2):
                    ps = psU.tile([128, 512], f32, tag="u", name="y")
                    for e in range(2):
                        nc.tensor.matmul(
                            ps[:],
                            g_sb[e][:, c2 * 128 : (c2 + 1) * 128],
                            ut[e][:],
                            start=(e == 0),
                            stop=(e == 1),
                        )
                    y_sb = y_pool.tile([128, 512], f32, tag="y", name="y")
                    nc.vector.tensor_tensor(y_sb[:], ps[:], rbc[:], ALU.mult)
                    nc.sync.dma_start(out_d[c2 * 128 : (c2 + 1) * 128, cols : cols + 512], y_sb[:])

            # ---- prologue: weights + batch-0 LN/transpose/T^T, pipelined ----
            nc.gpsimd.dma_start(m_sb[0][:], m_d[0:128, :])
            nc.gpsimd.dma_start(m_sb[1][:], m_d[128:256, :])
            nc.gpsimd.dma_start(g_sb[0][:], g_d[0:128, :])
            nc.gpsimd.dma_start(g_sb[1][:], g_d[128:256, :])
            warm(16)

            for t in range(8):
                ln_stats(t)
            ln_finish8(0)
            for t in range(8, 16):
                ln_stats(t)
                ln_norm(t - 8)
                tr(t - 8, nc.scalar if t % 2 else nc.sync)
            ln_finish8(1)
            for t in range(8, 16):
                ln_norm(t)
                tr(t, nc.scalar if t % 2 else nc.sync)
            tT_group(0)
            tT_group(1)
            warm(12)

            def mk(f, *a):
                return lambda: f(*a)

            def norm_tr(t):
                ln_norm(t)
                tr(t, nc.sync)

            # batch-1 prep woven into batch-0's S-phase matmul streams
            s_extras = {
                (0, 0): [mk(tT_group, 2)] + [mk(ln_stats, t) for t in range(16, 24)],
                (0, 1): [mk(tT_group, 3)]
                + [mk(ln_stats, t) for t in range(24, 32)]
                + [mk(ln_finish8, 2)],
                (0, 2): [mk(norm_tr, t) for t in range(16, 24)] + [mk(ln_finish8, 3)],
                (0, 3): [mk(norm_tr, t) for t in range(24, 32)] + [mk(tT_group, 4)],
                (1, 0): [mk(tT_group, 5)],
                (1, 1): [mk(tT_group, 6)],
                (1, 2): [mk(tT_group, 7)],
            }

            pending_y = None
            for b in range(B):
                for ch in range(NCH):
                    extras = ([pending_y] if pending_y else []) + list(
                        s_extras.get((b, ch), ())
                    )
                    phase_s(b, ch, extras)
                    phase_rsum()
                    phase_u(b, ch)
                    pending_y = mk(phase_y, b, ch, state["ut"], state["rbc"])
            pending_y()

    nc.compile()
    return nc


def get_nc():
    if "nc" not in _CACHE:
        _CACHE["nc"] = _build()
    return _CACHE["nc"]


def make_in_maps(x, gamma, Wq, Wk, Wv, Wo):
    bf = ml_dtypes.bfloat16
    gp = 1.0 + gamma.astype(np.float64)
    x_flat = np.ascontiguousarray(x.reshape(N_TOK, D).astype(np.float32))
    Wq = Wq.astype(np.float64)
    Wk = Wk.astype(np.float64)
    Wv = Wv.astype(np.float64)
    Wo = Wo.astype(np.float64)
    in_maps = []
    for h in range(HEADS):
        sl = slice(h * DH, (h + 1) * DH)
        M = SCALE * (gp[:, None] * Wq[sl].T) @ (Wk[sl] * gp[None, :])
        G = (gp[:, None] * Wv[sl].T) @ Wo[:, sl].T
        in_maps.append(
            {
                "x": x_flat,
                "m": np.ascontiguousarray(M.astype(bf)),
                "g": np.ascontiguousarray(G.astype(bf)),
            }
        )
    return in_maps


def kernel(x, gamma, Wq, Wk, Wv, Wo):
    from concourse import bass_utils

    x, gamma, Wq, Wk, Wv, Wo = (
        np.asarray(a) for a in (x, gamma, Wq, Wk, Wv, Wo)
    )
    nc = get_nc()
    in_maps = make_in_maps(x, gamma, Wq, Wk, Wv, Wo)
    res = bass_utils.run_bass_kernel_spmd(
        nc, in_maps, core_ids=list(range(HEADS))
    )
    acc = np.zeros((D, N_TOK), np.float32)
    for h in range(HEADS):
        acc += res.results[h]["outT"]
    return np.ascontiguousarray(acc.T).reshape(B, N_SEQ, D).astype(np.float32)


# revision 14
# speedup vs baseline: 5.8806x; 1.4741x over previous
"""Bass/Tile TRN2 kernel for nn_Attention_12704513261709 (low-rank factored).

Per-head dim (2048) >> model dim (256), so fold each head's weight pairs
into 256x256 matrices on the host:
  S_h = xn @ M_h @ xn^T    M_h = SCALE * diag(1+g) Wq_h^T Wk_h diag(1+g)
  Y_h = softmax(S_h) @ xn @ G_h    G_h = diag(1+g) Wv_h^T Wo_h^T
This cuts matmul FLOPs ~8.9x vs materializing q/k/v. Each of the 8 cores
computes one head over both batches; host sums the per-head partials.

Perf design (the PE p-state ramp runs at 1.2 GHz until ~3us of continuous
busy, 2.4 GHz after, and any stall resets it — so the matmul stream must
never wait):
 - xn transposes are XBAR DMA-transposes via a DRAM round-trip (few big
   DMAs, no PE/DVE work).
 - S^T tiles are computed in PAIRS into [128,1024] 2-bank PSUM tiles, one
   exp per pair (halves ACT instruction overhead).
 - U/Y of chunk q are deferred into chunk q+1's S-phase slots so the PE
   interleaves S pairs with U/Y chains while ACT exps trail behind.
 - softmax rowsum: contiguous bf16 add-ladder on DVE + GpSimd partition
   all-reduce; the final scale is a DVE divide (no reciprocal on the
   critical path).
 - LN sqrt/recip batched per 8 tiles (no ACT Sqrt/Exp table thrash).
"""

import numpy as np
import ml_dtypes

B = 2
N_SEQ = 2048
N_TOK = B * N_SEQ  # 4096
D = 256
HEADS = 8
INNER = 16384
DH = INNER // HEADS  # 2048
SCALE = 64 ** (-0.5)
EPS = 1e-5

TT = N_SEQ // 128  # 16 key tiles per batch
NCH = N_SEQ // 512  # 4 query chunks of 512 per batch
NTILE = N_TOK // 128  # 32 token tiles
NPAIR = TT // 2  # 8 S-tile pairs per chunk

_CACHE = {}


def _build():
    from concourse import bacc, bass_isa
    import concourse.tile as tile
    import concourse.mybir as mybir

    f32 = mybir.dt.float32
    bf16 = mybir.dt.bfloat16
    AF = mybir.ActivationFunctionType
    ALU = mybir.AluOpType

    nc = bacc.Bacc("TRN2", target_bir_lowering=False, debug=False, num_devices=8)

    x_d = nc.dram_tensor("x", [N_TOK, D], f32, kind="ExternalInput").ap()
    m_d = nc.dram_tensor("m", [D, D], bf16, kind="ExternalInput").ap()
    g_d = nc.dram_tensor("g", [D, D], bf16, kind="ExternalInput").ap()
    out_d = nc.dram_tensor("outT", [D, N_TOK], f32, kind="ExternalOutput").ap()

    with tile.TileContext(nc) as tc:
        with (
            tc.tile_pool(name="singles", bufs=1) as singles,
            tc.tile_pool(name="xt", bufs=6) as xt_pool,
            tc.tile_pool(name="lns", bufs=4) as lns_pool,
            tc.tile_pool(name="big", bufs=1) as big,
            tc.tile_pool(name="pt", bufs=2) as pt_pool,
            tc.tile_pool(name="ut", bufs=2) as ut_pool,
            tc.tile_pool(name="lad", bufs=1) as lad_pool,
            tc.tile_pool(name="rsum", bufs=2) as rsum_pool,
            tc.tile_pool(name="ystage", bufs=2) as y_pool,
            tc.tile_pool(name="dram", bufs=1, space="DRAM") as dram_pool,
            tc.tile_pool(name="psA", bufs=2, space="PSUM") as psA,
            tc.tile_pool(name="psUY", bufs=2, space="PSUM") as psUY,
        ):
            eps_t = singles.tile([128, 1], f32)
            nc.vector.memset(eps_t, EPS)
            dummy_w = singles.tile([128, 128], bf16)
            nc.vector.memset(dummy_w, 0.0)
            dummy_r = singles.tile([128, 512], bf16)
            nc.vector.memset(dummy_r, 0.0)

            def warm(n):
                for _ in range(n):
                    ps = psUY.tile([128, 1024], f32, tag="u", name="hamwarm")
                    nc.tensor.matmul(ps[:, :512], dummy_w[:], dummy_r[:], start=True, stop=True)

            m_sb = [big.tile([128, D], bf16, tag=f"m{c}", name=f"m{c}") for c in range(2)]
            g_sb = [big.tile([128, D], bf16, tag=f"g{c}", name=f"g{c}") for c in range(2)]
            xn_nat = big.tile([128, NTILE * D], bf16, tag="xn", name="xn")
            xnT = big.tile([128, 2, N_TOK], bf16, tag="xnT", name="xnT")
            tT = big.tile([128, 2, N_TOK], bf16, tag="tT", name="tT")
            mv_all = big.tile([128, NTILE, 2], f32, tag="mv", name="mv")
            rstd_all = big.tile([128, NTILE], f32, tag="rstd", name="rstd")
            xn_dram = dram_pool.tile([N_TOK, D], bf16, tag="xnd", name="xnd")

            state = {}

            def load_x4(g):
                """one 512-token slab of x -> SBUF [128, 4, 256]."""
                x4 = xt_pool.tile([128, 4, D], f32, tag="x4", name="x4")
                nc.sync.dma_start(
                    x4[:], x_d[g * 512 : (g + 1) * 512, :].rearrange("(t p) d -> p t d", p=128)
                )
                state[f"x4_{g}"] = x4

            def ln_stats(t):
                x_t = state[f"x4_{t // 4}"][:, t % 4, :]
                stats = lns_pool.tile([128, nc.vector.BN_STATS_DIM], f32, tag="st", name="st")
                nc.vector.bn_stats(stats[:], x_t)
                nc.vector.bn_aggr(mv_all[:, t, :], stats[:])

            def stats4(g):
                for t in range(4 * g, 4 * g + 4):
                    ln_stats(t)

            def ln_finish8(g8):
                """sqrt+reciprocal for one 8-tile group (batched)."""
                std8 = lns_pool.tile([128, 8], f32, tag="std8", name="std8")
                nc.scalar.activation(
                    std8[:], mv_all[:, g8 * 8 : (g8 + 1) * 8, 1], func=AF.Sqrt,
                    bias=eps_t[:], scale=1.0,
                )
                nc.vector.reciprocal(rstd_all[:, g8 * 8 : (g8 + 1) * 8], std8[:])

            def ln_norm(t):
                nc.vector.tensor_scalar(
                    xn_nat[:, t * D : (t + 1) * D],
                    state[f"x4_{t // 4}"][:, t % 4, :],
                    scalar1=mv_all[:, t, 0:1],
                    scalar2=rstd_all[:, t : t + 1],
                    op0=ALU.subtract,
                    op1=ALU.mult,
                )

            def store_g(g):
                """xn 512-token slab -> DRAM scratch (for XBAR transpose)."""
                nc.sync.dma_start(
                    xn_dram[g * 512 : (g + 1) * 512, :].rearrange("(t p) d -> p t d", p=128),
                    xn_nat[:, 4 * g * D : (4 * g + 4) * D].rearrange("p (t d) -> p t d", t=4),
                )

            def tload_g(g):
                """DRAM scratch -> xnT via XBAR DMA transpose."""
                for c in range(2):
                    nc.sync.dma_start_transpose(
                        xnT[:, c, g * 512 : (g + 1) * 512],
                        xn_dram[g * 512 : (g + 1) * 512, c * 128 : (c + 1) * 128],
                    )

            def norm_store_tload(g):
                for t in range(4 * g, 4 * g + 4):
                    ln_norm(t)
                store_g(g)
                tload_g(g)

            def tT_group(g):
                """T^T columns for one 512-token group, both d2-halves in one
                2-bank PSUM tile, one copy out."""
                ps = psUY.tile([128, 1024], f32, tag="u", name="tT")
                for c2 in range(2):
                    for c1 in range(2):
                        nc.tensor.matmul(
                            ps[:, c2 * 512 : (c2 + 1) * 512],
                            m_sb[c1][:, c2 * 128 : (c2 + 1) * 128],
                            xnT[:, c1, g * 512 : (g + 1) * 512],
                            start=(c1 == 0),
                            stop=(c1 == 1),
                        )
                nc.vector.tensor_copy(
                    tT[:, :, g * 512 : (g + 1) * 512],
                    ps.rearrange("p (c q) -> p c q", c=2),
                )

            def phase_s(b, ch, extras=()):
                """S^T pairs + exp for one 512-query chunk; extras[i] runs
                after pair i (deferred U/Y of the previous chunk, T^T groups,
                batch-1 LN prep)."""
                cols = b * N_SEQ + ch * 512
                pt_big = pt_pool.tile([128, TT * 512], bf16, tag="pt", name="pt")
                state["pt"] = pt_big
                for p in range(NPAIR):
                    ps = psA.tile([128, 1024], f32, tag="s", name="s")
                    for kk in range(2):
                        t = 2 * p + kk
                        for c in range(2):
                            nc.tensor.matmul(
                                ps[:, kk * 512 : (kk + 1) * 512],
                                xnT[:, c, b * N_SEQ + t * 128 : b * N_SEQ + (t + 1) * 128],
                                tT[:, c, cols : cols + 512],
                                start=(c == 0),
                                stop=(c == 1),
                            )
                    nc.scalar.activation(
                        pt_big[:, p * 1024 : (p + 1) * 1024], ps[:], func=AF.Exp
                    )
                    if p < len(extras):
                        extras[p]()
                for j in range(NPAIR, len(extras)):
                    extras[j]()

            def phase_rsum():
                pt_big = state["pt"]
                r1 = lad_pool.tile([128, 4096], bf16, tag="r1", name="r1")
                nc.vector.tensor_tensor(r1[:], pt_big[:, :4096], pt_big[:, 4096:], ALU.add)
                r2 = lad_pool.tile([128, 2048], bf16, tag="r2", name="r2")
                nc.vector.tensor_tensor(r2[:], r1[:, :2048], r1[:, 2048:], ALU.add)
                r3 = lad_pool.tile([128, 1024], bf16, tag="r3", name="r3")
                nc.vector.tensor_tensor(r3[:], r2[:, :1024], r2[:, 1024:], ALU.add)
                r4 = lad_pool.tile([128, 512], f32, tag="r4", name="r4")
                nc.vector.tensor_tensor(r4[:], r3[:, :512], r3[:, 512:], ALU.add)
                ra = rsum_pool.tile([128, 512], f32, tag="ra", name="ra")
                nc.gpsimd.partition_all_reduce(
                    ra[:], r4[:], channels=128, reduce_op=bass_isa.ReduceOp.add
                )
                state["ra"] = ra

            def mku_segs(b, ch, pt_big):
                """Deferred U-phase for chunk (b,ch): 4 PE segments (8 matmuls
                each, e0/e1 chains into the two halves of one 2-bank PSUM
                tile) + one copy-out thunk."""
                holder = {}

                def seg(e, half):
                    def run():
                        if "ps" not in holder:
                            holder["ps"] = psUY.tile([128, 1024], f32, tag="u", name="u")
                        ps = holder["ps"]
                        for t in range(8 * half, 8 * half + 8):
                            nc.tensor.matmul(
                                ps[:, e * 512 : (e + 1) * 512],
                                xn_nat[:, (b * TT + t) * D + e * 128 : (b * TT + t) * D + (e + 1) * 128],
                                pt_big[:, t * 512 : (t + 1) * 512],
                                start=(t == 0),
                                stop=(t == TT - 1),
                            )
                    return run

                def copy_out():
                    ut = ut_pool.tile([128, 1024], bf16, tag="ut", name="ut")
                    nc.vector.tensor_copy(ut[:], holder["ps"][:])
                    holder["ut"] = ut

                return [seg(0, 0), seg(0, 1), seg(1, 0), seg(1, 1), copy_out], holder

            def mky(b, ch, holder, ra):
                """Deferred Y-phase for chunk (b,ch): 4 matmuls into one
                2-bank PSUM tile, one DVE divide, one output DMA."""

                def run():
                    cols = b * N_SEQ + ch * 512
                    ut = holder["ut"]
                    ps = psUY.tile([128, 1024], f32, tag="u", name="y")
                    for c2 in range(2):
                        for e in range(2):
                            nc.tensor.matmul(
                                ps[:, c2 * 512 : (c2 + 1) * 512],
                                g_sb[e][:, c2 * 128 : (c2 + 1) * 128],
                                ut[:, e * 512 : (e + 1) * 512],
                                start=(e == 0),
                                stop=(e == 1),
                            )
                    rbc = y_pool.tile([128, 512], f32, tag="rbc", name="rbc")
                    nc.vector.reciprocal(rbc[:], ra[:])
                    y_sb = y_pool.tile([128, 1024], f32, tag="y", name="y")
                    for c2 in range(2):
                        nc.vector.tensor_tensor(
                            y_sb[:, c2 * 512 : (c2 + 1) * 512],
                            ps[:, c2 * 512 : (c2 + 1) * 512],
                            rbc[:],
                            ALU.mult,
                        )
                    nc.sync.dma_start(
                        out_d[0:256, cols : cols + 512].rearrange("(c p) q -> p c q", p=128),
                        y_sb.rearrange("p (c q) -> p c q", c=2),
                    )

                return run

            def mk(f, *a):
                return lambda: f(*a)

            # ---- prologue: weights + batch-0 LN/transpose + first T^T ----
            nc.gpsimd.dma_start(m_sb[0][:], m_d[0:128, :])
            nc.gpsimd.dma_start(m_sb[1][:], m_d[128:256, :])
            nc.gpsimd.dma_start(g_sb[0][:], g_d[0:128, :])
            nc.gpsimd.dma_start(g_sb[1][:], g_d[128:256, :])
            warm(10)

            for g in range(4):
                load_x4(g)
            stats4(0)
            stats4(1)
            ln_finish8(0)
            norm_store_tload(0)
            stats4(2)
            norm_store_tload(1)
            stats4(3)
            ln_finish8(1)
            norm_store_tload(2)
            norm_store_tload(3)
            tT_group(0)
            warm(8)

            # batch-1 prep + deferred U/Y woven into S-phase slots
            s_extras = {
                (0, 0): [mk(tT_group, 1), mk(load_x4, 4), mk(load_x4, 5),
                         mk(stats4, 4), mk(stats4, 5)],
                (0, 1): [mk(tT_group, 2), mk(load_x4, 6), mk(load_x4, 7),
                         mk(stats4, 6), mk(stats4, 7), mk(ln_finish8, 2)],
                (0, 2): [mk(tT_group, 3), mk(ln_finish8, 3),
                         mk(norm_store_tload, 4), mk(norm_store_tload, 5)],
                (0, 3): [mk(tT_group, 4), mk(norm_store_tload, 6),
                         mk(norm_store_tload, 7)],
                (1, 0): [mk(tT_group, 5)],
                (1, 1): [mk(tT_group, 6)],
                (1, 2): [mk(tT_group, 7)],
            }

            deferred = []  # thunks from the previous chunk (U segs, copy, Y)
            for b in range(B):
                for ch in range(NCH):
                    prep = s_extras.get((b, ch), ())
                    # slot layout: [tT/prep0, us0, us1, us2, us3+copy, prep…, Y]
                    extras = []
                    if prep:
                        extras.append(prep[0])
                    extras.extend(deferred[:-1])  # 4 U segs + ut copy
                    extras.extend(prep[1:])
                    if deferred:
                        extras.append(deferred[-1])  # Y last, after spacing
                    phase_s(b, ch, extras)
                    phase_rsum()
                    segs, holder = mku_segs(b, ch, state["pt"])
                    deferred = segs + [mky(b, ch, holder, state["ra"])]
            for th in deferred:
                th()

    nc.compile()
    return nc


def get_nc():
    if "nc" not in _CACHE:
        _CACHE["nc"] = _build()
    return _CACHE["nc"]


def make_in_maps(x, gamma, Wq, Wk, Wv, Wo):
    bf = ml_dtypes.bfloat16
    gp = 1.0 + gamma.astype(np.float64)
    x_flat = np.ascontiguousarray(x.reshape(N_TOK, D).astype(np.float32))
    Wq = Wq.astype(np.float64)
    Wk = Wk.astype(np.float64)
    Wv = Wv.astype(np.float64)
    Wo = Wo.astype(np.float64)
    in_maps = []
    for h in range(HEADS):
        sl = slice(h * DH, (h + 1) * DH)
        M = SCALE * (gp[:, None] * Wq[sl].T) @ (Wk[sl] * gp[None, :])
        G = (gp[:, None] * Wv[sl].T) @ Wo[:, sl].T
        in_maps.append(
            {
                "x": x_flat,
                "m": np.ascontiguousarray(M.astype(bf)),
                "g": np.ascontiguousarray(G.astype(bf)),
            }
        )
    return in_maps


def kernel(x, gamma, Wq, Wk, Wv, Wo):
    from concourse import bass_utils

    x, gamma, Wq, Wk, Wv, Wo = (
        np.asarray(a) for a in (x, gamma, Wq, Wk, Wv, Wo)
    )
    nc = get_nc()
    in_maps = make_in_maps(x, gamma, Wq, Wk, Wv, Wo)
    res = bass_utils.run_bass_kernel_spmd(
        nc, in_maps, core_ids=list(range(HEADS))
    )
    acc = np.zeros((D, N_TOK), np.float32)
    for h in range(HEADS):
        acc += res.results[h]["outT"]
    return np.ascontiguousarray(acc.T).reshape(B, N_SEQ, D).astype(np.float32)


# revision 15
# speedup vs baseline: 6.0314x; 1.0256x over previous
"""Bass/Tile TRN2 kernel for nn_Attention_12704513261709 (low-rank factored).

Per-head dim (2048) >> model dim (256), so fold each head's weight pairs
into 256x256 matrices on the host:
  S_h = xn @ M_h @ xn^T    M_h = SCALE * diag(1+g) Wq_h^T Wk_h diag(1+g)
  Y_h = softmax(S_h) @ xn @ G_h    G_h = diag(1+g) Wv_h^T Wo_h^T
This cuts matmul FLOPs ~8.9x vs materializing q/k/v. Each of the 8 cores
computes one head over both batches; host sums the per-head partials.

Perf design. The PE p-state ramp (1.2 GHz until ~3us of continuous busy,
2.4 GHz after; stalls reset it) and the ~166ns non-overlapped SBUF access
latency paid by any matmul that carries a semaphore wait mean the matmul
stream must be both gap-free and wait-free:
 - xn transposes are XBAR DMA-transposes via a DRAM round-trip.
 - xn/xnT/tT live in per-512-token-group tiles so dependency tracking is
   group-granular (one whole-tensor tile would make early readers wait on
   all later writers).
 - S^T tiles are computed in pairs into [128,1024] 2-bank PSUM tiles, one
   exp per pair; U/Y of chunk q are deferred into chunk q+1's S-phase
   slots so the PE interleaves while ACT exps trail.
 - U consumes P^T k-slices in reverse production order: its first matmul
   waits the newest exp event and every later wait is elided as redundant.
 - softmax rowsum: contiguous bf16 add-ladder on DVE + GpSimd partition
   all-reduce; reciprocal+scale sit at the end of the deferred Y slot.
 - LN sqrt/recip batched per 8 tiles (no ACT Sqrt/Exp table thrash).
"""

import numpy as np
import ml_dtypes

B = 2
N_SEQ = 2048
N_TOK = B * N_SEQ  # 4096
D = 256
HEADS = 8
INNER = 16384
DH = INNER // HEADS  # 2048
SCALE = 64 ** (-0.5)
EPS = 1e-5

TT = N_SEQ // 128  # 16 key tiles per batch
NCH = N_SEQ // 512  # 4 query chunks of 512 per batch
NG = N_TOK // 512  # 8 512-token groups
NPAIR = TT // 2  # 8 S-tile pairs per chunk

_CACHE = {}


def _build():
    from concourse import bacc, bass_isa
    import concourse.tile as tile
    import concourse.mybir as mybir

    f32 = mybir.dt.float32
    bf16 = mybir.dt.bfloat16
    AF = mybir.ActivationFunctionType
    ALU = mybir.AluOpType

    nc = bacc.Bacc("TRN2", target_bir_lowering=False, debug=False, num_devices=8)

    x_d = nc.dram_tensor("x", [N_TOK, D], f32, kind="ExternalInput").ap()
    m_d = nc.dram_tensor("m", [D, D], bf16, kind="ExternalInput").ap()
    g_d = nc.dram_tensor("g", [D, D], bf16, kind="ExternalInput").ap()
    out_d = nc.dram_tensor("outT", [D, N_TOK], f32, kind="ExternalOutput").ap()

    with tile.TileContext(nc) as tc:
        with (
            tc.tile_pool(name="singles", bufs=1) as singles,
            tc.tile_pool(name="xt", bufs=6) as xt_pool,
            tc.tile_pool(name="lns", bufs=4) as lns_pool,
            tc.tile_pool(name="big", bufs=1) as big,
            tc.tile_pool(name="pt", bufs=2) as pt_pool,
            tc.tile_pool(name="ut", bufs=2) as ut_pool,
            tc.tile_pool(name="lad", bufs=1) as lad_pool,
            tc.tile_pool(name="rsum", bufs=2) as rsum_pool,
            tc.tile_pool(name="ystage", bufs=2) as y_pool,
            tc.tile_pool(name="dram", bufs=1, space="DRAM") as dram_pool,
            tc.tile_pool(name="psA", bufs=2, space="PSUM") as psA,
            tc.tile_pool(name="psUY", bufs=2, space="PSUM") as psUY,
        ):
            eps_t = singles.tile([128, 1], f32)
            nc.vector.memset(eps_t, EPS)
            dummy_w = singles.tile([128, 128], bf16)
            nc.vector.memset(dummy_w, 0.0)
            dummy_r = singles.tile([128, 512], bf16)
            nc.vector.memset(dummy_r, 0.0)

            def warm(n):
                for _ in range(n):
                    ps = psUY.tile([128, 1024], f32, tag="u", name="hamwarm")
                    nc.tensor.matmul(ps[:, :512], dummy_w[:], dummy_r[:], start=True, stop=True)

            m_sb = [big.tile([128, D], bf16, tag=f"m{c}", name=f"m{c}") for c in range(2)]
            g_sb = [big.tile([128, D], bf16, tag=f"g{c}", name=f"g{c}") for c in range(2)]
            # per-512-token-group tiles (group-granular dependency tracking)
            xng = [big.tile([128, 4 * D], bf16, tag=f"xng{g}", name=f"xng{g}") for g in range(NG)]
            xnTg = [big.tile([128, 2, 512], bf16, tag=f"xnTg{g}", name=f"xnTg{g}") for g in range(NG)]
            tTg = [big.tile([128, 2, 512], bf16, tag=f"tTg{g}", name=f"tTg{g}") for g in range(NG)]
            mv_all = big.tile([128, 32, 2], f32, tag="mv", name="mv")
            rstd_all = big.tile([128, 32], f32, tag="rstd", name="rstd")
            xn_dram = dram_pool.tile([N_TOK, D], bf16, tag="xnd", name="xnd")

            state = {}

            def load_x4(g):
                x4 = xt_pool.tile([128, 4, D], f32, tag="x4", name="x4")
                nc.sync.dma_start(
                    x4[:], x_d[g * 512 : (g + 1) * 512, :].rearrange("(t p) d -> p t d", p=128)
                )
                state[f"x4_{g}"] = x4

            def ln_stats(t):
                x_t = state[f"x4_{t // 4}"][:, t % 4, :]
                stats = lns_pool.tile([128, nc.vector.BN_STATS_DIM], f32, tag="st", name="st")
                nc.vector.bn_stats(stats[:], x_t)
                nc.vector.bn_aggr(mv_all[:, t, :], stats[:])

            def stats4(g):
                for t in range(4 * g, 4 * g + 4):
                    ln_stats(t)

            def ln_finish8(g8):
                std8 = lns_pool.tile([128, 8], f32, tag="std8", name="std8")
                nc.scalar.activation(
                    std8[:], mv_all[:, g8 * 8 : (g8 + 1) * 8, 1], func=AF.Sqrt,
                    bias=eps_t[:], scale=1.0,
                )
                nc.vector.reciprocal(rstd_all[:, g8 * 8 : (g8 + 1) * 8], std8[:])

            def norm_store_tload(g):
                """LN-normalize group g, store to DRAM scratch, XBAR
                transpose-load back into xnTg[g]."""
                for t in range(4 * g, 4 * g + 4):
                    nc.vector.tensor_scalar(
                        xng[g][:, (t % 4) * D : (t % 4 + 1) * D],
                        state[f"x4_{g}"][:, t % 4, :],
                        scalar1=mv_all[:, t, 0:1],
                        scalar2=rstd_all[:, t : t + 1],
                        op0=ALU.subtract,
                        op1=ALU.mult,
                    )
                nc.sync.dma_start(
                    xn_dram[g * 512 : (g + 1) * 512, :].rearrange("(t p) d -> p t d", p=128),
                    xng[g].rearrange("p (t d) -> p t d", t=4),
                )
                for c in range(2):
                    nc.sync.dma_start_transpose(
                        xnTg[g][:, c, :],
                        xn_dram[g * 512 : (g + 1) * 512, c * 128 : (c + 1) * 128],
                    )

            def tT_group(g):
                ps = psUY.tile([128, 1024], f32, tag="u", name="tT")
                for c2 in range(2):
                    for c1 in range(2):
                        nc.tensor.matmul(
                            ps[:, c2 * 512 : (c2 + 1) * 512],
                            m_sb[c1][:, c2 * 128 : (c2 + 1) * 128],
                            xnTg[g][:, c1, :],
                            start=(c1 == 0),
                            stop=(c1 == 1),
                        )
                nc.vector.tensor_copy(tTg[g][:], ps.rearrange("p (c q) -> p c q", c=2))

            def phase_s(b, ch, extras=()):
                """S^T pairs + exp for one 512-query chunk; extras[i] runs
                after pair i."""
                pt_big = pt_pool.tile([128, TT * 512], bf16, tag="pt", name="pt")
                state["pt"] = pt_big
                tt = tTg[b * NCH + ch]
                for p in range(NPAIR):
                    ps = psA.tile([128, 1024], f32, tag="s", name="s")
                    for kk in range(2):
                        t = 2 * p + kk
                        g = b * NCH + t // 4
                        for c in range(2):
                            nc.tensor.matmul(
                                ps[:, kk * 512 : (kk + 1) * 512],
                                xnTg[g][:, c, (t % 4) * 128 : (t % 4 + 1) * 128],
                                tt[:, c, :],
                                start=(c == 0),
                                stop=(c == 1),
                            )
                    nc.scalar.activation(
                        pt_big[:, p * 1024 : (p + 1) * 1024], ps[:], func=AF.Exp
                    )
                    if p < len(extras):
                        th = extras[p]
                        if th is not None:
                            th()
                for j in range(NPAIR, len(extras)):
                    th = extras[j]
                    if th is not None:
                        th()

            def phase_rsum():
                pt_big = state["pt"]
                r1 = lad_pool.tile([128, 4096], bf16, tag="r1", name="r1")
                nc.vector.tensor_tensor(r1[:], pt_big[:, :4096], pt_big[:, 4096:], ALU.add)
                r2 = lad_pool.tile([128, 2048], bf16, tag="r2", name="r2")
                nc.vector.tensor_tensor(r2[:], r1[:, :2048], r1[:, 2048:], ALU.add)
                r3 = lad_pool.tile([128, 1024], bf16, tag="r3", name="r3")
                nc.vector.tensor_tensor(r3[:], r2[:, :1024], r2[:, 1024:], ALU.add)
                r4 = lad_pool.tile([128, 512], f32, tag="r4", name="r4")
                nc.vector.tensor_tensor(r4[:], r3[:, :512], r3[:, 512:], ALU.add)
                ra = rsum_pool.tile([128, 512], f32, tag="ra", name="ra")
                nc.gpsimd.partition_all_reduce(
                    ra[:], r4[:], channels=128, reduce_op=bass_isa.ReduceOp.add
                )
                state["ra"] = ra

            def mku_segs(b, ch, pt_big):
                """Deferred U-phase: 4 PE segments. P^T k-slices are consumed
                newest-exp-first so only the first matmul carries a wait; the
                e0/e1 chains land in the two halves of one 2-bank PSUM tile,
                each copied out as soon as its chain stops."""
                holder = {}

                def seg(e, half):
                    def run():
                        if "ps" not in holder:
                            holder["ps"] = psUY.tile([128, 1024], f32, tag="u", name="u")
                        ps = holder["ps"]
                        for t in range(8 * (1 - half) + 7, 8 * (1 - half) - 1, -1):
                            nc.tensor.matmul(
                                ps[:, e * 512 : (e + 1) * 512],
                                xng[b * NCH + t // 4][:, (t % 4) * D + e * 128 : (t % 4) * D + (e + 1) * 128],
                                pt_big[:, t * 512 : (t + 1) * 512],
                                start=(t == TT - 1),
                                stop=(t == 0),
                            )
                        if half == 1:
                            ut = ut_pool.tile([128, 512], bf16, tag=f"ut{e}", name=f"ut{e}")
                            nc.vector.tensor_copy(ut[:], ps[:, e * 512 : (e + 1) * 512])
                            holder[f"ut{e}"] = ut
                    return run

                return [seg(0, 0), seg(0, 1), seg(1, 0), seg(1, 1)], holder

            def mky(b, ch, holder, ra):
                def run():
                    cols = b * N_SEQ + ch * 512
                    ps = psUY.tile([128, 1024], f32, tag="u", name="y")
                    for c2 in range(2):
                        for e in range(2):
                            nc.tensor.matmul(
                                ps[:, c2 * 512 : (c2 + 1) * 512],
                                g_sb[e][:, c2 * 128 : (c2 + 1) * 128],
                                holder[f"ut{e}"][:],
                                start=(e == 0),
                                stop=(e == 1),
                            )
                    rbc = y_pool.tile([128, 512], f32, tag="rbc", name="rbc")
                    nc.vector.reciprocal(rbc[:], ra[:])
                    y_sb = y_pool.tile([128, 1024], f32, tag="y", name="y")
                    for c2 in range(2):
                        nc.vector.tensor_tensor(
                            y_sb[:, c2 * 512 : (c2 + 1) * 512],
                            ps[:, c2 * 512 : (c2 + 1) * 512],
                            rbc[:],
                            ALU.mult,
                        )
                    nc.sync.dma_start(
                        out_d[0:256, cols : cols + 512].rearrange("(c p) q -> p c q", p=128),
                        y_sb.rearrange("p (c q) -> p c q", c=2),
                    )
                return run

            def mk(f, *a):
                return lambda: f(*a)

            # ---- prologue ----
            nc.gpsimd.dma_start(m_sb[0][:], m_d[0:128, :])
            nc.gpsimd.dma_start(m_sb[1][:], m_d[128:256, :])
            nc.gpsimd.dma_start(g_sb[0][:], g_d[0:128, :])
            nc.gpsimd.dma_start(g_sb[1][:], g_d[128:256, :])
            warm(20)

            for g in range(4):
                load_x4(g)
            stats4(0)
            stats4(1)
            ln_finish8(0)
            norm_store_tload(0)
            stats4(2)
            norm_store_tload(1)
            stats4(3)
            ln_finish8(1)
            norm_store_tload(2)
            norm_store_tload(3)
            tT_group(0)
            warm(4)

            preps = {
                (0, 0): [mk(load_x4, 4), mk(load_x4, 5), mk(stats4, 4), mk(stats4, 5)],
                (0, 1): [mk(load_x4, 6), mk(load_x4, 7), mk(stats4, 6), mk(stats4, 7),
                         mk(ln_finish8, 2)],
                (0, 2): [mk(ln_finish8, 3), mk(norm_store_tload, 4), mk(norm_store_tload, 5)],
                (0, 3): [mk(norm_store_tload, 6), mk(norm_store_tload, 7)],
            }

            deferred = None  # (u segs, y thunk) from the previous chunk
            for b in range(B):
                for ch in range(NCH):
                    q = b * NCH + ch
                    prep = list(preps.get((b, ch), []))
                    extras = []
                    if q + 1 < NG:
                        extras.append(mk(tT_group, q + 1))
                    if deferred is not None:
                        segs, yth = deferred
                        extras.extend(segs)  # slots 1-4
                        extras.extend(prep[:2] if prep else [None, None])  # 5-6
                        extras.append(yth)  # slot 7
                        extras.extend(prep[2:])
                    else:
                        extras.extend(prep)
                    phase_s(b, ch, extras)
                    phase_rsum()
                    segs, holder = mku_segs(b, ch, state["pt"])
                    deferred = (segs, mky(b, ch, holder, state["ra"]))
            segs, yth = deferred
            for th in segs:
                th()
            yth()

    nc.compile()
    return nc


def get_nc():
    if "nc" not in _CACHE:
        _CACHE["nc"] = _build()
    return _CACHE["nc"]


def make_in_maps(x, gamma, Wq, Wk, Wv, Wo):
    bf = ml_dtypes.bfloat16
    gp = 1.0 + gamma.astype(np.float64)
    x_flat = np.ascontiguousarray(x.reshape(N_TOK, D).astype(np.float32))
    Wq = Wq.astype(np.float64)
    Wk = Wk.astype(np.float64)
    Wv = Wv.astype(np.float64)
    Wo = Wo.astype(np.float64)
    in_maps = []
    for h in range(HEADS):
        sl = slice(h * DH, (h + 1) * DH)
        M = SCALE * (gp[:, None] * Wq[sl].T) @ (Wk[sl] * gp[None, :])
        G = (gp[:, None] * Wv[sl].T) @ Wo[:, sl].T
        in_maps.append(
            {
                "x": x_flat,
                "m": np.ascontiguousarray(M.astype(bf)),
                "g": np.ascontiguousarray(G.astype(bf)),
            }
        )
    return in_maps


def kernel(x, gamma, Wq, Wk, Wv, Wo):
    from concourse import bass_utils

    x, gamma, Wq, Wk, Wv, Wo = (
        np.asarray(a) for a in (x, gamma, Wq, Wk, Wv, Wo)
    )
    nc = get_nc()
    in_maps = make_in_maps(x, gamma, Wq, Wk, Wv, Wo)
    res = bass_utils.run_bass_kernel_spmd(
        nc, in_maps, core_ids=list(range(HEADS))
    )
    acc = np.zeros((D, N_TOK), np.float32)
    for h in range(HEADS):
        acc += res.results[h]["outT"]
    return np.ascontiguousarray(acc.T).reshape(B, N_SEQ, D).astype(np.float32)
